# revision 1
# baseline (speedup 1.0000x reference)
"""CTAttention Trainium2 kernel.

Full-input contract: kernel(**inputs) takes the unsharded numpy inputs and
returns the full [total, C] output. Internally: data-parallel over the batch
axis B=8 across 8 NeuronCores (attention is independent per batch element);
qkv/proj weights replicated; ragged scatter/gather bookkeeping on the host.

Per-core dataflow (batch b, dense 1024 windows, 8 heads, head_dim 32):
  X^T[256,1024] -> Q^T/K^T (fp32r, channel-on-partition) and V[kpos,ch] (bf16)
  attention runs 4 heads (one group) at a time, software-pipelined:
    scores: S^T = per-head K=32 matmuls, 4-way row-packed on the PE array
            (fp32r, one 32-row strip per head), into two [128,1024] psums
    exp:    one ScalarE Exp per psum tile with the key-padding mask as a
            per-partition bias (masked scores underflow to exactly 0, so no
            row-max pass is needed); output P^T in bf16
    PV:     bf16 matmuls col-tiled across the 4 PE column strips; an extra
            M=1 all-ones matmul per head accumulates the softmax denominator
            into a spare psum row of the pair accumulator
  normalization: denominators -> 1/x via ScalarE Ln then Exp(-x) (both live in
  the same ACT table set, so no table switches), broadcast to all 32-row
  strips with a single K=4 selector matmul, one DVE multiply per half.
  Output projection in fp32r from the strip-assembled O^T.

Exact algebraic simplifications vs the reference:
  - K bias dropped (softmax is invariant to per-query constant shifts)
  - V bias folded into the proj bias (softmax weights sum to 1)
  - head-dim scale folded into the exp's input scale

Environment workarounds (this walrus build): at most one sem wait per
instruction (waits hoisted onto injected NOPs), fp32/fp32r matmuls require
dst partition base 0, no gpsimd extended instructions, no custom DVE ops.
"""

import sys

if "/opt/trn_rl_repo" not in sys.path:
    sys.path.insert(0, "/opt/trn_rl_repo")

import numpy as np

B = 8
C = 256
H = 8
HD = 32
MAXW = 1024
SCALE = HD ** -0.5
NEG_THRESH = -1e8  # mask values below this count as fully masked

_cached = {}


def _build_nc():
    import bass_rust
    import concourse.bass as bass
    import concourse.tile as tile
    import concourse.mybir as mybir
    from concourse.vector_clock import ScopedClock

    # ---- workaround: this walrus build accepts at most ONE sem wait per
    # instruction ("Too many sync wait commands" in setupSyncWait). Tile
    # attaches multi-sem waits freely. Split: hoist all but the last wait of
    # every committed instruction onto injected same-engine NOPs, and split
    # the final drain the same way.
    _ctr = [0]

    def _hoist_excess_waits(tc_self, inst, orig_add):
        si = inst.sync_info
        if si is not None:
            waits = list(si.on_wait or [])
            if len(waits) > 1:
                for w in waits[:-1]:
                    _ctr[0] += 1
                    nop = mybir.InstNoOp(name=f"waitsplit-{_ctr[0]}")
                    nop.engine = inst.engine
                    nop.sync_info = bass_rust.SyncInfo(on_wait=[w], on_update=[])
                    orig_add(tc_self, nop)
                si.on_wait = waits[-1:]
        orig_add(tc_self, inst)

    if not getattr(tile.TileContext, "_waitsplit_patched", False):
        _orig_add_instruction = tile.TileContext._add_instruction

        def _split_add_instruction(self, inst):
            _hoist_excess_waits(self, inst, _orig_add_instruction)

        tile.TileContext._add_instruction = _split_add_instruction

        def _patched_drain_and_barrier(self, tick_clock, wait_clock):
            nc = self.nc
            d0 = nc.sync.drain()
            wait_clock.add_sem_waits(
                d0.ins, ScopedClock({None: tick_clock.global_clock})
            )
            si = d0.ins.sync_info
            waits = list(si.on_wait) if si is not None else []
            if len(waits) > 1:
                si.on_wait = waits[0:1]
                for w in waits[1:]:
                    dk = nc.sync.drain()
                    dk.ins.sync_info = bass_rust.SyncInfo(on_wait=[w], on_update=[])
            nc.all_engine_barrier()
            assert self.sems is not None
            popped = nc._tile_sem_poison_stack.pop()
            assert popped is self._sem_poison
            nc.clear_and_free_semaphores(list(self.sems.allocated().values()))
            nc.all_engine_barrier()

        tile.TileContext._drain_and_barrier = _patched_drain_and_barrier
        tile.TileContext._waitsplit_patched = True

    dt = mybir.dt
    f32 = dt.float32
    f32r = dt.float32r
    AF = mybir.ActivationFunctionType

    nc = bass.Bass(
        "TRN2",
        target_bir_lowering=False,
        debug=False,
        num_devices=1,
        enable_asserts=False,
    )

    xt_d = nc.dram_tensor("xt", [128, 2048], f32r, kind="ExternalInput").ap()
    qw_d = nc.dram_tensor("qw", [128, 1536], f32r, kind="ExternalInput").ap()
    qb_d = nc.dram_tensor("qb", [128, 2], f32, kind="ExternalInput").ap()
    pw_d = nc.dram_tensor("pw", [128, 512], f32r, kind="ExternalInput").ap()
    pb_d = nc.dram_tensor("pb", [128, 2], f32, kind="ExternalInput").ap()
    mask_d = nc.dram_tensor("mask", [128, 8], f32, kind="ExternalInput").ap()
    onesb_d = nc.dram_tensor("onesb", [128, 4], dt.bfloat16, kind="ExternalInput").ap()
    sel_d = nc.dram_tensor("sel", [4, 128], f32r, kind="ExternalInput").ap()
    yt_d = nc.dram_tensor("yt", [128, 2048], f32, kind="ExternalOutput").ap()

    bf16 = dt.bfloat16

    with tile.TileContext(nc) as tc:
        with (
            tc.tile_pool(name="const", bufs=1) as const_pool,
            tc.tile_pool(name="big", bufs=1) as big_pool,
            tc.tile_pool(name="pt", bufs=6) as pt_pool,
            tc.tile_pool(name="stage", bufs=4) as stage_pool,
            tc.tile_pool(name="norm", bufs=2) as norm_pool,
            tc.tile_pool(name="ps_s4a", bufs=1, space="PSUM") as ps_s4a,
            tc.tile_pool(name="ps_s4b", bufs=1, space="PSUM") as ps_s4b,
            tc.tile_pool(name="ps_ot", bufs=2, space="PSUM") as ps_ot,
        ):
            xt = const_pool.tile([128, 2048], f32r, tag="xt")
            qw = const_pool.tile([128, 1536], f32r, tag="qw")
            qb = const_pool.tile([128, 2], f32, tag="qb")
            pw = const_pool.tile([128, 512], f32r, tag="pw")
            pb = const_pool.tile([128, 2], f32, tag="pb")
            mask = const_pool.tile([128, 8], f32, tag="mask")
            onesb = const_pool.tile([128, 4], bf16, tag="onesb")
            sel = const_pool.tile([4, 128], f32r, tag="sel")

            nc.gpsimd.dma_start(sel[:], sel_d)
            warm = const_pool.tile([4, 128], f32, tag="warm")
            nc.scalar.activation(warm[:], sel[:], AF.Exp, scale=0.0)
            for q4 in range(4):
                eng = nc.sync if q4 % 2 == 0 else nc.scalar
                eng.dma_start(
                    xt[:, 512 * q4 : 512 * (q4 + 1)],
                    xt_d[:, 512 * q4 : 512 * (q4 + 1)],
                )
            nc.sync.dma_start(qw[:, 0:768], qw_d[:, 0:768])
            nc.scalar.dma_start(qw[:, 768:1536], qw_d[:, 768:1536])
            nc.gpsimd.dma_start(qb[:], qb_d)
            nc.gpsimd.dma_start(pw[:], pw_d)
            nc.gpsimd.dma_start(pb[:], pb_d)
            nc.gpsimd.dma_start(mask[:], mask_d)
            nc.gpsimd.dma_start(onesb[:], onesb_d)

            qt = big_pool.tile([128, 2048], f32r, tag="qt")
            kt = big_pool.tile([128, 2048], f32r, tag="kt")
            va = big_pool.tile([128, 8, 8, 32], bf16, tag="va")  # [part, kpos_blk, head, head_dim]
            otf = big_pool.tile([128, 2048], f32r, tag="otf")
            ytile = big_pool.tile([128, 2048], f32, tag="ytile")

            # ---------- qkv projections ----------
            # V first (PV needs all of it), then the Q/K tiles needed by the
            # first head group; the rest is emitted between the groups so the
            # PE fills ACT-bound gaps.
            def qk_tile(m, chunks=(0, 1), pool=None, ptag="ot"):
                pool = pool if pool is not None else ps_ot
                for c in chunks:
                    ps = pool.tile([128, 512], f32, tag=ptag, name=f"qk{m}{c}")
                    for t in range(2):
                        nc.tensor.matmul(
                            ps[:],
                            qw[:, 768 * t + 128 * m : 768 * t + 128 * (m + 1)],
                            xt[:, 1024 * t + 512 * c : 1024 * t + 512 * (c + 1)],
                            start=(t == 0),
                            stop=(t == 1),
                        )
                    if m < 2:
                        nc.vector.tensor_scalar_add(
                            qt[:, 1024 * m + 512 * c : 1024 * m + 512 * (c + 1)],
                            ps[:],
                            qb[:, m : m + 1],
                        )
                    else:
                        nc.vector.tensor_copy(
                            kt[:, 1024 * (m - 2) + 512 * c : 1024 * (m - 2) + 512 * (c + 1)],
                            ps[:],
                        )

            # V: out[kpos_block, cv] in bf16 (no bias; folded into proj bias)
            def v_tile(j, pool=None, ptag="ot"):
                pool = pool if pool is not None else ps_ot
                ps = pool.tile([128, 256], f32, tag=ptag, name=f"v{j}")
                for t in range(2):
                    nc.tensor.matmul(
                        ps[:],
                        xt[:, 1024 * t + 128 * j : 1024 * t + 128 * (j + 1)],
                        qw[:, 768 * t + 512 : 768 * t + 768],
                        start=(t == 0),
                        stop=(t == 1),
                    )
                nc.vector.tensor_copy(
                    va[:, j, :, :],
                    ps[:].rearrange("p (h d) -> p h d", d=32),
                )

            qk_tile(0)
            qk_tile(2)
            v_tile(0)
            v_tile(1)

            # ---------- attention: 4 heads (one group) at a time ----------
            # scores: 4-way row-packed fp32r matmuls, two [128,1024] psum tiles
            # per step (double-buffered so ScalarE exps run back-to-back), one
            # exp per tile -> bf16 P^T, then 8 bf16 PV/rowsum matmuls col-tiled
            # across the 4 PE column strips.
            # O^T strips: head hh at psum rows 32*hh of its pair psum
            # (pair 0 = hh 0,1 rows 0-63; pair 1 = hh 2,3 rows 64-127);
            # denominator rows: hh -> (64, 96, 0, 32).
            saved = {}
            for grp in range(2):
                ov0 = ps_ot.tile([128, 1024], f32, tag="ot", name=f"ov0_{grp}")
                ov1 = ps_ot.tile([128, 1024], f32, tag="ot", name=f"ov1_{grp}")

                def emit_scores(j, c):
                    s4a = ps_s4a.tile([128, 1024], f32, tag="s4a", name=f"s4a{grp}{j}{c}")
                    s4b = ps_s4b.tile([128, 1024], f32, tag="s4b", name=f"s4b{grp}{j}{c}")
                    for hh in range(4):
                        s4 = s4a if hh < 2 else s4b
                        base = 32 * hh
                        nc.tensor.matmul(
                            s4[:, 512 * (hh % 2) : 512 * (hh % 2 + 1)],
                            kt[base : base + 32,
                               1024 * grp + 128 * j : 1024 * grp + 128 * (j + 1)],
                            qt[base : base + 32,
                               1024 * grp + 512 * c : 1024 * grp + 512 * (c + 1)],
                            start=True,
                            stop=True,
                            tile_position=(base, 0),
                        )
                    pta = pt_pool.tile([128, 1024], bf16, tag="pt", name=f"pta{grp}{j}{c}")
                    ptb = pt_pool.tile([128, 1024], bf16, tag="pt", name=f"ptb{grp}{j}{c}")
                    nc.scalar.activation(
                        pta[:], s4a[:], AF.Exp, bias=mask[:, j : j + 1], scale=SCALE,
                    )
                    nc.scalar.activation(
                        ptb[:], s4b[:], AF.Exp, bias=mask[:, j : j + 1], scale=SCALE,
                    )
                    return pta, ptb

                def emit_pv(pta, ptb, j, c):
                    sj = (j == 0)
                    ej = (j == 7)
                    for hh in range(4):
                        h = 4 * grp + hh
                        ov = ov0 if hh < 2 else ov1
                        pt = pta if hh < 2 else ptb
                        vpos = 32 * hh
                        nc.tensor.matmul(
                            ov[vpos : vpos + 32, 512 * c : 512 * (c + 1)],
                            va[:, j, h, :],
                            pt[:, 512 * (hh % 2) : 512 * (hh % 2 + 1)],
                            start=sj,
                            stop=ej,
                            tile_position=(0, vpos),
                        )
                    for hh in range(4):
                        ov = ov0 if hh < 2 else ov1
                        pt = pta if hh < 2 else ptb
                        spos = (64, 96, 0, 32)[hh]
                        nc.tensor.matmul(
                            ov[spos : spos + 1, 512 * c : 512 * (c + 1)],
                            onesb[:, 0:1],
                            pt[:, 512 * (hh % 2) : 512 * (hh % 2 + 1)],
                            start=sj,
                            stop=ej,
                            tile_position=(0, spos),
                        )

                iters = [(j, c) for j in range(8) for c in range(2)]
                pend = None
                for idx in range(len(iters) + 1):
                    if idx < len(iters):
                        j, c = iters[idx]
                        cur = (*emit_scores(j, c), j, c)
                        if grp == 0 and idx == 0:
                            # V projections and the remaining Q/K tiles fill the
                            # first exp latencies; emitted before any PV so the
                            # ov accumulators don't yet hold the psum slots
                            for jj in range(2, 6):
                                v_tile(jj)
                        if grp == 0 and idx == 1:
                            v_tile(6)
                            v_tile(7)
                            qk_tile(1)
                            qk_tile(3)
                    else:
                        cur = None
                    if pend is not None:
                        emit_pv(*pend)
                    pend = cur

                # ---- staging + reciprocal for this group (bc/mult deferred) ----
                st0 = stage_pool.tile([128, 1024], f32, tag="st", name=f"st0_{grp}")
                st1 = stage_pool.tile([128, 1024], f32, tag="st", name=f"st1_{grp}")
                nc.vector.tensor_copy(st0[:], ov0[:])
                if grp == 1:
                    nc.scalar.copy(st1[:], ov1[:])
                else:
                    nc.vector.tensor_copy(st1[:], ov1[:])
                se4 = norm_pool.tile([4, 1024], f32, tag="se4", name=f"se4_{grp}")
                nc.sync.dma_start(se4[0:1, :], st0[64:65, :])
                nc.scalar.dma_start(se4[1:2, :], st0[96:97, :])
                nc.sync.dma_start(se4[2:3, :], st1[0:1, :])
                nc.scalar.dma_start(se4[3:4, :], st1[32:33, :])
                ln4 = norm_pool.tile([4, 1024], f32, tag="ln4", name=f"ln4_{grp}")
                rc4 = norm_pool.tile([4, 1024], f32r, tag="rc4", name=f"rc4_{grp}")
                nc.scalar.activation(ln4[:], se4[:], AF.Ln)
                nc.scalar.activation(rc4[:], ln4[:], AF.Exp, scale=-1.0)
                saved[grp] = (st0, st1, rc4)

            # ---- deferred normalization: broadcast 1/denominator and scale ----
            for grp in range(2):
                st0, st1, rc4 = saved[grp]
                bpool = ps_s4a if grp == 0 else ps_s4b
                bc = bpool.tile([128, 1024], f32, tag=("s4a" if grp == 0 else "s4b"), name=f"bc{grp}")
                for c in range(2):
                    nc.tensor.matmul(
                        bc[:, 512 * c : 512 * (c + 1)],
                        sel[:, :],
                        rc4[:, 512 * c : 512 * (c + 1)],
                        start=True,
                        stop=True,
                    )
                nc.vector.tensor_mul(
                    otf[0:64, 1024 * grp : 1024 * (grp + 1)], st0[0:64, :], bc[0:64, :]
                )
                nc.vector.tensor_mul(
                    otf[64:128, 1024 * grp : 1024 * (grp + 1)], st1[64:128, :], bc[64:128, :]
                )

            # ---------- output projection ----------
            for m in range(2):
                for c in range(2):
                    ps = ps_ot.tile([128, 512], f32, tag="ot")
                    for t in range(2):
                        nc.tensor.matmul(
                            ps[:],
                            pw[:, 256 * t + 128 * m : 256 * t + 128 * (m + 1)],
                            otf[:, 1024 * t + 512 * c : 1024 * t + 512 * (c + 1)],
                            start=(t == 0),
                            stop=(t == 1),
                        )
                    nc.vector.tensor_scalar_add(
                        ytile[:, 1024 * m + 512 * c : 1024 * m + 512 * (c + 1)],
                        ps[:],
                        pb[:, m : m + 1],
                    )
                (nc.sync if m == 0 else nc.scalar).dma_start(
                    yt_d[:, 1024 * m : 1024 * (m + 1)],
                    ytile[:, 1024 * m : 1024 * (m + 1)],
                )

    return nc


def _get_nc():
    if "nc" not in _cached:
        _cached["nc"] = _build_nc()
    return _cached["nc"]


def _pack_per_partition(a2d):
    """[2*128, F] -> [128, 2*F] with tile t at cols F*t."""
    n, f = a2d.shape
    t = n // 128
    return np.ascontiguousarray(
        a2d.reshape(t, 128, f).transpose(1, 0, 2).reshape(128, t * f)
    )


def _prepare(carrier_tokens, ct_mask, batch_num_windows, qkv_w, qkv_b, proj_w, proj_b):
    """Host-side bookkeeping: ragged->padded scatter, weight packing.
    Returns (in_maps, ctx) where ctx carries what postprocessing needs."""
    carrier_tokens = np.asarray(carrier_tokens, dtype=np.float32)
    ct_mask = np.asarray(ct_mask, dtype=np.float32)
    lens = np.asarray(batch_num_windows).astype(np.int64)
    qkv_w = np.asarray(qkv_w, dtype=np.float32)
    qkv_b = np.asarray(qkv_b, dtype=np.float32)
    proj_w = np.asarray(proj_w, dtype=np.float32)
    proj_b = np.asarray(proj_b, dtype=np.float32)

    total = carrier_tokens.shape[0]

    # ragged -> padded bookkeeping (mirrors the reference's scatter semantics:
    # OOB scatter indices dropped, OOB gather indices clipped)
    offsets = np.concatenate([[0], np.cumsum(lens)])
    tok = np.arange(total)
    b_id = np.searchsorted(offsets[1:], tok, side="right")
    w_id = tok - offsets[np.minimum(b_id, B)]
    flat_idx = b_id * MAXW + w_id
    valid = flat_idx < B * MAXW
    padded = np.zeros((B * MAXW, C), np.float32)
    padded[flat_idx[valid]] = carrier_tokens[valid]
    padded = padded.reshape(B, MAXW, C)

    mask_col = np.ascontiguousarray(ct_mask[:, 0, :])  # [B, MAXW]

    # host-side exact weight transforms
    pw_perm = proj_w
    pb_eff = qkv_b[2 * C : 3 * C] @ proj_w + proj_b

    qw_packed = _pack_per_partition(qkv_w)                      # [128, 1536]
    qb_packed = np.ascontiguousarray(qkv_b[0:C].reshape(2, 128).T)
    pw_packed = _pack_per_partition(pw_perm)                    # [128, 512]
    pb_packed = np.ascontiguousarray(pb_eff.reshape(2, 128).T)

    import ml_dtypes
    onesb_arr = np.ones((128, 4), ml_dtypes.bfloat16)
    sel_arr = np.zeros((4, 128), np.float32)
    for k in range(4):
        sel_arr[k, 32 * k : 32 * (k + 1)] = 1.0
    in_maps = []
    for b in range(B):
        xt = _pack_per_partition(padded[b].T)                   # [128, 2048]
        mb = np.ascontiguousarray(mask_col[b].reshape(8, 128).T)
        in_maps.append(
            {
                "xt": xt,
                "qw": qw_packed,
                "qb": qb_packed,
                "pw": pw_packed,
                "pb": pb_packed,
                "mask": mb,
                "onesb": onesb_arr,
                "sel": sel_arr,
            }
        )

    ctx = {
        "flat_idx": flat_idx,
        "mask_col": mask_col,
        "padded": padded,
        "qkv_w": qkv_w,
        "qkv_b": qkv_b,
        "proj_w": proj_w,
        "proj_b": proj_b,
    }
    return in_maps, ctx


def _postprocess(results, ctx):
    """Per-core outputs -> full ragged output (gather + degenerate-row fix)."""
    flat_idx = ctx["flat_idx"]
    mask_col = ctx["mask_col"]
    padded = ctx["padded"]
    qkv_w, qkv_b = ctx["qkv_w"], ctx["qkv_b"]
    proj_w, proj_b = ctx["proj_w"], ctx["proj_b"]

    y_pad = np.empty((B, MAXW, C), np.float32)
    for b in range(B):
        yt = results[b]["yt"]                                   # [128, 2048]
        y_t = yt.reshape(128, 2, MAXW).transpose(1, 0, 2).reshape(C, MAXW)
        y_pad[b] = y_t.T
    y_flat = y_pad.reshape(B * MAXW, C)
    gather_idx = np.clip(flat_idx, 0, B * MAXW - 1)
    out = y_flat[gather_idx]

    # degenerate rows: gathered positions whose key mask is fully masked.
    # The reference's softmax (with max-subtraction) gives uniform weights
    # there; our exp underflows to 0/0. Recompute those rows exactly.
    row_b = np.minimum(gather_idx // MAXW, B - 1)
    degenerate_batches = [b for b in range(B) if np.all(mask_col[b] < NEG_THRESH)]
    for b in degenerate_batches:
        rows = np.nonzero(row_b == b)[0]
        if rows.size == 0:
            continue
        vmat = padded[b] @ qkv_w[:, 2 * C : 3 * C] + qkv_b[2 * C : 3 * C]
        mean_v = vmat.mean(axis=0)  # uniform attention, same for all heads
        fix = mean_v @ proj_w + proj_b
        out[rows] = fix.astype(np.float32)

    return np.ascontiguousarray(out.astype(np.float32))


def run_device(in_maps, **spmd_kwargs):
    from concourse import bass_utils

    nc = _get_nc()
    return bass_utils.run_bass_kernel_spmd(
        nc, in_maps, core_ids=list(range(B)), **spmd_kwargs
    )


def kernel(carrier_tokens, ct_mask, batch_num_windows, qkv_w, qkv_b, proj_w, proj_b):
    in_maps, ctx = _prepare(
        carrier_tokens, ct_mask, batch_num_windows, qkv_w, qkv_b, proj_w, proj_b
    )
    res = run_device(in_maps, trace=False)
    return _postprocess(res.results, ctx)



# revision 8
# speedup vs baseline: 1.0233x; 1.0233x over previous
"""CTAttention Trainium2 kernel — ragged-rebalanced, ACT-bound schedule.

Full-input contract: kernel(**inputs) takes the unsharded numpy inputs and
returns the full [total, C] output.

Sharding: instead of one batch element per core (padded to 1024 windows),
the ragged batches are split into 512-query chunks (queries never cross a
batch boundary) and the 15 real chunks are load-balanced across 8 cores.
Every core runs the same SPMD schedule: one "big" slot (8 key-blocks) and
one "small" slot (SMALL_KB key-blocks, 6 for the expected lengths). This
cuts score/exp/PV work per core from 16 key-block iterations per head
group to 8+SMALL_KB, which matters because the kernel is bound by the
ScalarE (ACT) engine doing the softmax exps.

Per-core dataflow (2 head groups of 4 heads, head_dim 32):
  K/V are projected for the (<=2) batches the core's slots reference,
  Q for the core's 1024 slot queries. Scores run 4 heads at a time as
  4-way row-packed fp32r matmuls into two [128,1024] psum tiles; one
  ScalarE Exp per tile (key-padding mask as per-partition bias; masked
  scores underflow to exactly 0) produces P^T in bf16; PV + softmax
  denominator are bf16 matmuls into per-slot [128,512] psum accumulators.
  ScalarE does nothing but the exps (denominator reciprocals run on the
  DVE, staging copies on DVE/GpSimd, all DMAs on sync/vector/gpsimd), so
  the steady state is one exp after another.

Exact algebraic simplifications vs the reference:
  - K bias dropped (softmax is invariant to per-query constant shifts)
  - V bias folded into the proj bias (softmax weights sum to 1)
  - head-dim scale folded into the exp's input scale

Environment workarounds (this walrus build): at most one sem wait per
instruction (waits hoisted onto injected NOPs), fp32/fp32r matmuls require
dst partition base 0, matmul psum dst <= 1 bank (512 fp32 cols).
"""

import math
import sys

if "/opt/trn_rl_repo" not in sys.path:
    sys.path.insert(0, "/opt/trn_rl_repo")

import numpy as np

B = 8
C = 256
H = 8
HD = 32
MAXW = 1024
CHUNK = 512
BIGKB_MAX = MAXW // 128  # 8
SCALE = HD ** -0.5
NEG_THRESH = -1e8  # mask values below this count as fully masked

_cached = {}


# ---------------------------------------------------------------------------
# schedule: ragged batches -> per-core (big slot, small slot) chunk assignment
# ---------------------------------------------------------------------------
class _Sched:
    pass


def _schedule(lens):
    lens = np.clip(np.asarray(lens, np.int64), 0, MAXW)
    chunks = []  # (b, q0, qlen, kb)
    for b in range(B):
        L = int(lens[b])
        if L <= 0:
            continue
        kb = (L + 127) // 128
        for q0 in range(0, L, CHUNK):
            chunks.append((b, q0, min(CHUNK, L - q0), kb))
    # stable sort by descending key-block count
    chunks.sort(key=lambda c: -c[3])
    bigs = chunks[:B]
    smalls = chunks[B:]
    assert len(smalls) <= B, "more than 16 query chunks cannot happen (lens<=1024)"
    big_kb = max([c[3] for c in bigs], default=1)
    small_kb = max([c[3] for c in smalls], default=1)
    s = _Sched()
    s.lens = lens
    s.bigs = bigs + [None] * (B - len(bigs))
    s.smalls = smalls + [None] * (B - len(smalls))
    s.big_kb = max(1, big_kb)
    s.small_kb = max(1, small_kb)
    return s


# ---------------------------------------------------------------------------
# device program
# ---------------------------------------------------------------------------
def _build_nc(big_kb, small_kb):
    import bass_rust
    import concourse.bass as bass
    import concourse.tile as tile
    import concourse.mybir as mybir
    from concourse.vector_clock import ScopedClock

    # ---- workaround: this walrus build accepts at most ONE sem wait per
    # instruction ("Too many sync wait commands" in setupSyncWait). Tile
    # attaches multi-sem waits freely. Split: hoist all but the last wait of
    # every committed instruction onto injected same-engine NOPs, and split
    # the final drain the same way.
    _ctr = [0]

    def _hoist_excess_waits(tc_self, inst, orig_add):
        si = inst.sync_info
        if si is not None:
            waits = list(si.on_wait or [])
            if len(waits) > 1:
                for w in waits[:-1]:
                    _ctr[0] += 1
                    nop = mybir.InstNoOp(name=f"waitsplit-{_ctr[0]}")
                    nop.engine = inst.engine
                    nop.sync_info = bass_rust.SyncInfo(on_wait=[w], on_update=[])
                    orig_add(tc_self, nop)
                si.on_wait = waits[-1:]
        orig_add(tc_self, inst)

    if not getattr(tile.TileContext, "_waitsplit_patched", False):
        _orig_add_instruction = tile.TileContext._add_instruction

        def _split_add_instruction(self, inst):
            _hoist_excess_waits(self, inst, _orig_add_instruction)

        tile.TileContext._add_instruction = _split_add_instruction

        def _patched_drain_and_barrier(self, tick_clock, wait_clock):
            nc = self.nc
            d0 = nc.sync.drain()
            wait_clock.add_sem_waits(
                d0.ins, ScopedClock({None: tick_clock.global_clock})
            )
            si = d0.ins.sync_info
            waits = list(si.on_wait) if si is not None else []
            if len(waits) > 1:
                si.on_wait = waits[0:1]
                for w in waits[1:]:
                    dk = nc.sync.drain()
                    dk.ins.sync_info = bass_rust.SyncInfo(on_wait=[w], on_update=[])
            nc.all_engine_barrier()
            assert self.sems is not None
            popped = nc._tile_sem_poison_stack.pop()
            assert popped is self._sem_poison
            nc.clear_and_free_semaphores(list(self.sems.allocated().values()))
            nc.all_engine_barrier()

        tile.TileContext._drain_and_barrier = _patched_drain_and_barrier
        tile.TileContext._waitsplit_patched = True

    dt = mybir.dt
    f32 = dt.float32
    f32r = dt.float32r
    bf16 = dt.bfloat16
    AF = mybir.ActivationFunctionType

    KBS = (big_kb, small_kb)
    KBTOT = big_kb + small_kb
    KT = KBTOT * 128  # kt/xk columns per channel-tile
    REG0 = big_kb * 128  # region-0 key columns

    nc = bass.Bass(
        "TRN2",
        target_bir_lowering=False,
        debug=False,
        num_devices=1,
        enable_asserts=False,
    )

    xq_d = nc.dram_tensor("xq", [128, 2048], f32r, kind="ExternalInput").ap()
    xk_d = nc.dram_tensor("xk", [128, 2 * KT], f32r, kind="ExternalInput").ap()
    qw_d = nc.dram_tensor("qw", [128, 1536], f32r, kind="ExternalInput").ap()
    qb_d = nc.dram_tensor("qb", [128, 2], f32, kind="ExternalInput").ap()
    pw_d = nc.dram_tensor("pw", [128, 512], f32r, kind="ExternalInput").ap()
    pb_d = nc.dram_tensor("pb", [128, 2], f32, kind="ExternalInput").ap()
    mask_d = nc.dram_tensor("mask", [128, KBTOT], f32, kind="ExternalInput").ap()
    onesb_d = nc.dram_tensor("onesb", [128, 4], bf16, kind="ExternalInput").ap()
    sel_d = nc.dram_tensor("sel", [4, 128], f32r, kind="ExternalInput").ap()
    yt_d = nc.dram_tensor("yt", [128, 2048], f32, kind="ExternalOutput").ap()

    with tile.TileContext(nc) as tc:
        with (
            tc.tile_pool(name="const", bufs=1) as const_pool,
            tc.tile_pool(name="big", bufs=1) as big_pool,
            tc.tile_pool(name="pt", bufs=20) as pt_pool,
            tc.tile_pool(name="stage", bufs=4) as stage_pool,
            tc.tile_pool(name="norm", bufs=2) as norm_pool,
            tc.tile_pool(name="ps_s4a", bufs=1, space="PSUM") as ps_s4a,
            tc.tile_pool(name="ps_s4b", bufs=1, space="PSUM") as ps_s4b,
            tc.tile_pool(name="ps_acc", bufs=2, space="PSUM") as ps_acc,
        ):
            xq = const_pool.tile([128, 2048], f32r, tag="xq")
            xk = const_pool.tile([128, 2 * KT], f32r, tag="xk")
            qw = const_pool.tile([128, 1536], f32r, tag="qw")
            qb = const_pool.tile([128, 2], f32, tag="qb")
            pw = const_pool.tile([128, 512], f32r, tag="pw")
            pb = const_pool.tile([128, 2], f32, tag="pb")
            mask = const_pool.tile([128, KBTOT], f32, tag="mask")
            onesb = const_pool.tile([128, 4], bf16, tag="onesb")
            sel = const_pool.tile([4, 128], f32r, tag="sel")

            # small consts on the gpsimd queue; ACT table warm-up off sel
            nc.gpsimd.dma_start(sel[:], sel_d)
            warm = const_pool.tile([4, 128], f32, tag="warm")
            nc.scalar.activation(warm[:], sel[:], AF.Exp, scale=0.0)
            nc.gpsimd.dma_start(mask[:], mask_d)
            nc.gpsimd.dma_start(onesb[:], onesb_d)
            nc.gpsimd.dma_start(qb[:], qb_d)
            nc.gpsimd.dma_start(pb[:], pb_d)

            # big inputs across three DGE queues, in first-use order. The
            # scalar queue is only used during the idle head (before the
            # first real exp issues).
            nc.sync.dma_start(qw[:, 0:768], qw_d[:, 0:768])
            nc.scalar.dma_start(qw[:, 768:1536], qw_d[:, 768:1536])
            nc.scalar.dma_start(xq[:, 1024:2048], xq_d[:, 1024:2048])
            for c0 in range(0, REG0, 512):
                ce = min(c0 + 512, REG0)
                nc.sync.dma_start(xk[:, c0:ce], xk_d[:, c0:ce])
                nc.gpsimd.dma_start(xk[:, KT + c0 : KT + ce], xk_d[:, KT + c0 : KT + ce])
            nc.sync.dma_start(xq[:, 0:1024], xq_d[:, 0:1024])
            for c0 in range(REG0, KT, 512):
                ce = min(c0 + 512, KT)
                nc.sync.dma_start(xk[:, c0:ce], xk_d[:, c0:ce])
                nc.gpsimd.dma_start(xk[:, KT + c0 : KT + ce], xk_d[:, KT + c0 : KT + ce])
            nc.gpsimd.dma_start(pw[:], pw_d)

            qt = big_pool.tile([128, 2048], f32r, tag="qt")
            kt = big_pool.tile([128, 2 * KT], f32r, tag="kt")
            va = big_pool.tile([128, KBTOT, 8, 32], bf16, tag="va")
            otf = big_pool.tile([128, 2048], f32r, tag="otf")
            ytile = big_pool.tile([128, 2048], f32, tag="ytile")

            _acc_flip = [0]

            def acc_tile(shape, dtype, name):
                tag = "acca" if _acc_flip[0] == 0 else "accb"
                _acc_flip[0] ^= 1
                return ps_acc.tile(shape, dtype, tag=tag, name=name)

            # ---------- projection scratch (emitted before any PV binds) ----
            def q_tile(m, c):
                ps = acc_tile([128, 512], f32, f"q{m}{c}")
                for t in range(2):
                    nc.tensor.matmul(
                        ps[:],
                        qw[:, 768 * t + 128 * m : 768 * t + 128 * (m + 1)],
                        xq[:, 1024 * t + 512 * c : 1024 * t + 512 * (c + 1)],
                        start=(t == 0),
                        stop=(t == 1),
                    )
                nc.vector.tensor_scalar_add(
                    qt[:, 1024 * m + 512 * c : 1024 * m + 512 * (c + 1)],
                    ps[:],
                    qb[:, m : m + 1],
                )

            def k_tile(m2, c0, w):
                ps = acc_tile([128, 512], f32, f"k{m2}{c0}")
                for t in range(2):
                    nc.tensor.matmul(
                        ps[0:128, 0:w],
                        qw[:, 768 * t + 128 * (2 + m2) : 768 * t + 128 * (3 + m2)],
                        xk[:, KT * t + c0 : KT * t + c0 + w],
                        start=(t == 0),
                        stop=(t == 1),
                    )
                nc.vector.tensor_copy(kt[:, KT * m2 + c0 : KT * m2 + c0 + w], ps[0:128, 0:w])

            def v_tile(j):
                ps = acc_tile([128, 512], f32, f"v{j}")
                for t in range(2):
                    nc.tensor.matmul(
                        ps[0:128, 0:256],
                        xk[:, KT * t + 128 * j : KT * t + 128 * (j + 1)],
                        qw[:, 768 * t + 512 : 768 * t + 768],
                        start=(t == 0),
                        stop=(t == 1),
                    )
                nc.vector.tensor_copy(
                    va[:, j, :, :],
                    ps[0:128, 0:256].rearrange("p (h d) -> p h d", d=32),
                )

            # per-iteration scratch emission plan (all before the first PV)
            kchunks = [(c0, min(512, KT - c0)) for c0 in range(0, KT, 512)]
            scratch = {None: [lambda: k_tile(0, *kchunks[0]), lambda: q_tile(0, 0)]}
            rest = []
            if len(kchunks) > 1:
                rest.append(lambda: k_tile(0, *kchunks[1]))
            rest += [lambda: v_tile(0), lambda: v_tile(1), lambda: q_tile(0, 1)]
            rest += [lambda kc=kc: k_tile(1, *kc) for kc in kchunks[:2]]
            rest += [lambda: v_tile(2), lambda: v_tile(3)]
            rest += [lambda kc=kc: k_tile(0, *kc) for kc in kchunks[2:]]
            rest += [lambda: v_tile(4), lambda: v_tile(5), lambda: q_tile(1, 0)]
            rest += [lambda kc=kc: k_tile(1, *kc) for kc in kchunks[2:]]
            rest += [lambda: q_tile(1, 1)]
            rest += [lambda j=j: v_tile(j) for j in range(6, KBTOT)]
            LAG = 5
            per = max(1, math.ceil(len(rest) / LAG))
            for i in range(LAG):
                scratch[i] = rest[i * per : (i + 1) * per]

            # ---------- attention loop ----------
            iters = [
                (g, s, j) for g in range(2) for s in range(2) for j in range(KBS[s])
            ]

            def emit_scores(g, s, j):
                jj = j + (big_kb if s == 1 else 0)
                s4a = ps_s4a.tile([128, 1024], f32, tag="s4a", name=f"s4a{g}{s}{j}")
                s4b = ps_s4b.tile([128, 1024], f32, tag="s4b", name=f"s4b{g}{s}{j}")
                for hh in range(4):
                    s4 = s4a if hh < 2 else s4b
                    base = 32 * hh
                    nc.tensor.matmul(
                        s4[:, 512 * (hh % 2) : 512 * (hh % 2 + 1)],
                        kt[base : base + 32, KT * g + 128 * jj : KT * g + 128 * (jj + 1)],
                        qt[base : base + 32, 1024 * g + 512 * s : 1024 * g + 512 * (s + 1)],
                        start=True,
                        stop=True,
                        tile_position=(base, 0),
                    )
                pta = pt_pool.tile([128, 1024], bf16, tag="pt", name=f"pta{g}{s}{j}")
                ptb = pt_pool.tile([128, 1024], bf16, tag="pt", name=f"ptb{g}{s}{j}")
                nc.scalar.activation(
                    pta[:], s4a[:], AF.Exp, bias=mask[:, jj : jj + 1], scale=SCALE
                )
                nc.scalar.activation(
                    ptb[:], s4b[:], AF.Exp, bias=mask[:, jj : jj + 1], scale=SCALE
                )
                return pta, ptb

            accs = {}  # (g, s) -> (ova, ovb)

            def emit_pv_den(g, s, j, pta, ptb):
                if j == 0:
                    ova = acc_tile([128, 512], f32, f"ova{g}{s}")
                    ovb = acc_tile([128, 512], f32, f"ovb{g}{s}")
                    accs[(g, s)] = (ova, ovb)
                ova, ovb = accs[(g, s)]
                jj = j + (big_kb if s == 1 else 0)
                sj = j == 0
                ej = j == KBS[s] - 1
                for hh in range(4):
                    ov = ova if hh < 2 else ovb
                    pt = pta if hh < 2 else ptb
                    vpos = 32 * hh
                    nc.tensor.matmul(
                        ov[vpos : vpos + 32, :],
                        va[:, jj, 4 * g + hh, :],
                        pt[:, 512 * (hh % 2) : 512 * (hh % 2 + 1)],
                        start=sj,
                        stop=ej,
                        tile_position=(0, vpos),
                    )
                for hh in range(4):
                    ov = ova if hh < 2 else ovb
                    pt = pta if hh < 2 else ptb
                    spos = (64, 96, 0, 32)[hh]
                    nc.tensor.matmul(
                        ov[spos : spos + 1, :],
                        onesb[:, 0:1],
                        pt[:, 512 * (hh % 2) : 512 * (hh % 2 + 1)],
                        start=sj,
                        stop=ej,
                        tile_position=(0, spos),
                    )

            def emit_norm(g, s):
                """Stage accumulators, broadcast reciprocals, write otf strips."""
                ova, ovb = accs.pop((g, s))
                sta = stage_pool.tile([128, 512], f32, tag="st", name=f"sta{g}{s}")
                stb = stage_pool.tile([128, 512], f32, tag="st", name=f"stb{g}{s}")
                nc.vector.tensor_copy(sta[:], ova[:])
                nc.vector.tensor_copy(stb[:], ovb[:])
                se4 = norm_pool.tile([4, 512], f32, tag="se4", name=f"se4{g}{s}")
                nc.gpsimd.dma_start(se4[0:1, :], sta[64:65, :])
                nc.gpsimd.dma_start(se4[1:2, :], sta[96:97, :])
                nc.gpsimd.dma_start(se4[2:3, :], stb[0:1, :])
                nc.gpsimd.dma_start(se4[3:4, :], stb[32:33, :])
                rc4 = norm_pool.tile([4, 512], f32r, tag="rc4", name=f"rc4{g}{s}")
                with nc.allow_low_precision(reason="f32r is 32-bit; PE-mode tag only"):
                    nc.vector.reciprocal(rc4[:], se4[:])
                bc = acc_tile([128, 512], f32, f"bc{g}{s}")
                nc.tensor.matmul(bc[:], sel[:], rc4[:], start=True, stop=True)
                col = 1024 * g + 512 * s
                nc.vector.tensor_mul(
                    otf[0:64, col : col + 512], sta[0:64, :], bc[0:64, :]
                )
                nc.vector.tensor_mul(
                    otf[64:128, col : col + 512], stb[64:128, :], bc[64:128, :]
                )

            def emit_proj(c):
                for m in range(2):
                    ps = acc_tile([128, 512], f32, f"y{m}{c}")
                    for t in range(2):
                        nc.tensor.matmul(
                            ps[:],
                            pw[:, 256 * t + 128 * m : 256 * t + 128 * (m + 1)],
                            otf[:, 1024 * t + 512 * c : 1024 * t + 512 * (c + 1)],
                            start=(t == 0),
                            stop=(t == 1),
                        )
                    col = 1024 * m + 512 * c
                    nc.vector.tensor_scalar_add(
                        ytile[:, col : col + 512], ps[:], pb[:, m : m + 1]
                    )
                    # keep ScalarE exp-only until the last exp has issued:
                    # proj(c=0) runs while group-1 exps are still streaming
                    eng = nc.sync if m == 0 else (nc.scalar if c == 1 else nc.gpsimd)
                    eng.dma_start(
                        yt_d[:, col : col + 512], ytile[:, col : col + 512]
                    )

            for t in scratch[None]:
                t()
            pts = {}
            for idx in range(len(iters) + LAG):
                if idx < len(iters):
                    pts[idx] = emit_scores(*iters[idx])
                    for t in scratch.get(idx, []):
                        t()
                lag_idx = idx - LAG
                if 0 <= lag_idx < len(iters):
                    g, s, j = iters[lag_idx]
                    emit_pv_den(g, s, j, *pts.pop(lag_idx))
                    if j == KBS[s] - 1:
                        emit_norm(g, s)
                        if g == 1:
                            emit_proj(s)

    return nc


def _get_nc(big_kb, small_kb):
    key = (big_kb, small_kb)
    if key not in _cached:
        _cached[key] = _build_nc(big_kb, small_kb)
    return _cached[key]


def _pack_per_partition(a2d):
    """[2*128, F] -> [128, 2*F] with tile t at cols F*t."""
    n, f = a2d.shape
    t = n // 128
    return np.ascontiguousarray(
        a2d.reshape(t, 128, f).transpose(1, 0, 2).reshape(128, t * f)
    )


def _prepare(carrier_tokens, ct_mask, batch_num_windows, qkv_w, qkv_b, proj_w, proj_b):
    """Host-side bookkeeping: chunk scheduling, gathers, weight packing."""
    carrier_tokens = np.asarray(carrier_tokens, dtype=np.float32)
    ct_mask = np.asarray(ct_mask, dtype=np.float32)
    lens_raw = np.asarray(batch_num_windows).astype(np.int64)
    qkv_w = np.asarray(qkv_w, dtype=np.float32)
    qkv_b = np.asarray(qkv_b, dtype=np.float32)
    proj_w = np.asarray(proj_w, dtype=np.float32)
    proj_b = np.asarray(proj_b, dtype=np.float32)

    total = carrier_tokens.shape[0]
    sched = _schedule(lens_raw)
    lens = sched.lens
    KBTOT = sched.big_kb + sched.small_kb
    KT = KBTOT * 128

    offsets = np.concatenate([[0], np.cumsum(lens_raw)])
    mask_col = np.ascontiguousarray(ct_mask[:, 0, :])  # [B, MAXW]

    # batch key-token matrices, zero-padded to MAXW
    xb = np.zeros((B, MAXW, C), np.float32)
    for b in range(B):
        L = int(min(lens[b], max(0, total - offsets[b])))
        if L > 0:
            xb[b, :L] = carrier_tokens[offsets[b] : offsets[b] + L]

    # host-side exact weight transforms
    pb_eff = qkv_b[2 * C : 3 * C] @ proj_w + proj_b
    qw_packed = _pack_per_partition(qkv_w)  # [128, 1536]
    qb_packed = np.ascontiguousarray(qkv_b[0:C].reshape(2, 128).T)
    pw_packed = _pack_per_partition(proj_w)  # [128, 512]
    pb_packed = np.ascontiguousarray(pb_eff.reshape(2, 128).T)

    import ml_dtypes

    onesb_arr = np.ones((128, 4), ml_dtypes.bfloat16)
    sel_arr = np.zeros((4, 128), np.float32)
    for k in range(4):
        sel_arr[k, 32 * k : 32 * (k + 1)] = 1.0

    in_maps = []
    for i in range(B):
        slots = (sched.bigs[i], sched.smalls[i])
        widths = (sched.big_kb * 128, sched.small_kb * 128)
        # queries: 512 per slot, zero-padded
        xq2 = np.zeros((1024, C), np.float32)
        # keys: region per slot
        xk2 = np.zeros((KT, C), np.float32)
        kmask = np.full((128, KBTOT), -1e9, np.float32)
        kbase = 0
        jbase = 0
        for si, sl in enumerate(slots):
            w = widths[si]
            if sl is not None:
                b, q0, qlen, kb = sl
                xq2[512 * si : 512 * si + qlen] = xb[b, q0 : q0 + qlen]
                reg = min(w, MAXW)
                xk2[kbase : kbase + reg] = xb[b, :reg]
                m = mask_col[b, :reg].reshape(-1, 128).T  # [128, reg/128]
                kmask[:, jbase : jbase + reg // 128] = m
            kbase += w
            jbase += w // 128
        in_maps.append(
            {
                "xq": _pack_per_partition(xq2.T),
                "xk": _pack_per_partition(xk2.T),
                "qw": qw_packed,
                "qb": qb_packed,
                "pw": pw_packed,
                "pb": pb_packed,
                "mask": np.ascontiguousarray(kmask),
                "onesb": onesb_arr,
                "sel": sel_arr,
            }
        )

    ctx = {
        "sched": sched,
        "offsets": offsets,
        "total": total,
        "mask_col": mask_col,
        "xb": xb,
        "qkv_w": qkv_w,
        "qkv_b": qkv_b,
        "proj_w": proj_w,
        "proj_b": proj_b,
    }
    return in_maps, ctx


def _host_zero_rows(ctx, batches):
    """Exact reference rows for padded-position gathers / fully-masked
    batches: attention output for each batch's padded (zero) query token,
    with the reference's max-subtracted softmax."""
    out = {}
    qkv_w, qkv_b = ctx["qkv_w"], ctx["qkv_b"]
    proj_w, proj_b = ctx["proj_w"], ctx["proj_b"]
    mask_col = ctx["mask_col"]
    for b in batches:
        xpad = ctx["xb"][b]  # [MAXW, C] zero-padded
        kmat = xpad @ qkv_w[:, C : 2 * C] + qkv_b[C : 2 * C]
        vmat = xpad @ qkv_w[:, 2 * C : 3 * C] + qkv_b[2 * C : 3 * C]
        qvec = qkv_b[0:C].reshape(H, HD) * SCALE
        kh = kmat.reshape(MAXW, H, HD)
        sc = np.einsum("hd,khd->hk", qvec, kh) + mask_col[b][None, :]
        sc = sc - sc.max(axis=1, keepdims=True)
        e = np.exp(sc)
        attn = e / e.sum(axis=1, keepdims=True)
        ct = np.einsum("hk,khd->hd", attn, vmat.reshape(MAXW, H, HD)).reshape(C)
        out[b] = ct @ proj_w + proj_b
    return out


def _postprocess(results, ctx):
    """Per-core outputs -> full ragged output."""
    sched = ctx["sched"]
    total = ctx["total"]
    offsets = ctx["offsets"]
    mask_col = ctx["mask_col"]

    y_pad = np.zeros((B * MAXW, C), np.float32)
    written = np.zeros(B * MAXW, bool)
    for i in range(B):
        yt = results[i]["yt"]  # [128, 2048]
        y_t = yt.reshape(128, 2, 1024).transpose(1, 0, 2).reshape(C, 1024)
        y_core = y_t.T  # [1024, C]
        for si, sl in enumerate((sched.bigs[i], sched.smalls[i])):
            if sl is None:
                continue
            b, q0, qlen, _ = sl
            y_pad[b * MAXW + q0 : b * MAXW + q0 + qlen] = y_core[
                512 * si : 512 * si + qlen
            ]
            written[b * MAXW + q0 : b * MAXW + q0 + qlen] = True

    # reference gather semantics (OOB scatter dropped, OOB gather clipped)
    tok = np.arange(total)
    b_id = np.searchsorted(offsets[1:], tok, side="right")
    w_id = tok - offsets[np.minimum(b_id, B)]
    flat_idx = b_id * MAXW + w_id
    gather_idx = np.clip(flat_idx, 0, B * MAXW - 1)
    out = y_pad[gather_idx]

    # rare exact fixes: gathers landing on never-computed padded rows, and
    # fully-masked batches (device exp gives 0/0 where the reference's
    # max-subtracted softmax gives uniform weights)
    degenerate = set(
        b for b in range(B) if np.all(mask_col[b] < NEG_THRESH)
    )
    bad = ~written[gather_idx]
    row_b = np.minimum(gather_idx // MAXW, B - 1)
    if degenerate:
        bad |= np.isin(row_b, list(degenerate))
    if bad.any():
        need = set(row_b[bad].tolist()) | degenerate
        fixes = _host_zero_rows(ctx, sorted(need))
        for b, fix in fixes.items():
            rows = np.nonzero(bad & (row_b == b))[0]
            out[rows] = fix.astype(np.float32)

    return np.ascontiguousarray(out.astype(np.float32))


def run_device(in_maps, **spmd_kwargs):
    from concourse import bass_utils

    nc = _cached["last_nc"]
    return bass_utils.run_bass_kernel_spmd(
        nc, in_maps, core_ids=list(range(B)), **spmd_kwargs
    )


def kernel(carrier_tokens, ct_mask, batch_num_windows, qkv_w, qkv_b, proj_w, proj_b):
    in_maps, ctx = _prepare(
        carrier_tokens, ct_mask, batch_num_windows, qkv_w, qkv_b, proj_w, proj_b
    )
    sched = ctx["sched"]
    _cached["last_nc"] = _get_nc(sched.big_kb, sched.small_kb)
    res = run_device(in_maps, trace=False)
    return _postprocess(res.results, ctx)


# revision 15
# speedup vs baseline: 1.0902x; 1.0654x over previous
"""CTAttention Trainium2 kernel — ragged-rebalanced, ACT-bound schedule.

Full-input contract: kernel(**inputs) takes the unsharded numpy inputs and
returns the full [total, C] output.

Sharding: instead of one batch element per core (padded to 1024 windows),
the ragged batches are split into 512-query chunks (queries never cross a
batch boundary) and the 15 real chunks are load-balanced across 8 cores.
Every core runs the same SPMD schedule: one "big" slot (8 key-blocks) and
one "small" slot (SMALL_KB key-blocks, 6 for the expected lengths). This
cuts score/exp/PV work per core from 16 key-block iterations per head
group to 8+SMALL_KB, which matters because the kernel is bound by the
ScalarE (ACT) engine doing the softmax exps.

Per-core dataflow (2 head groups of 4 heads, head_dim 32):
  K/V are projected for the (<=2) batches the core's slots reference,
  Q for the core's 1024 slot queries. Scores run 4 heads at a time as
  4-way row-packed fp32r matmuls into two [128,1024] psum tiles; one
  ScalarE Exp per tile (key-padding mask as per-partition bias; masked
  scores underflow to exactly 0) produces P^T in bf16; PV + softmax
  denominator are bf16 matmuls into per-slot [128,512] psum accumulators.
  ScalarE does nothing but the exps (denominator reciprocals run on the
  DVE, staging copies on DVE/GpSimd, all DMAs on sync/vector/gpsimd), so
  the steady state is one exp after another.

Exact algebraic simplifications vs the reference:
  - K bias dropped (softmax is invariant to per-query constant shifts)
  - V bias folded into the proj bias (softmax weights sum to 1)
  - head-dim scale folded into the exp's input scale

Environment workarounds (this walrus build): at most one sem wait per
instruction (waits hoisted onto injected NOPs), fp32/fp32r matmuls require
dst partition base 0, matmul psum dst <= 1 bank (512 fp32 cols).
"""

import math
import sys

if "/opt/trn_rl_repo" not in sys.path:
    sys.path.insert(0, "/opt/trn_rl_repo")

import numpy as np

B = 8
C = 256
H = 8
HD = 32
MAXW = 1024
CHUNK = 512
BIGKB_MAX = MAXW // 128  # 8
SCALE = HD ** -0.5
NEG_THRESH = -1e8  # mask values below this count as fully masked

_cached = {}


# ---------------------------------------------------------------------------
# schedule: ragged batches -> per-core (big slot, small slot) chunk assignment
# ---------------------------------------------------------------------------
class _Sched:
    pass


def _schedule(lens):
    lens = np.clip(np.asarray(lens, np.int64), 0, MAXW)
    chunks = []  # (b, q0, qlen, kb)
    for b in range(B):
        L = int(lens[b])
        if L <= 0:
            continue
        kb = (L + 127) // 128
        for q0 in range(0, L, CHUNK):
            chunks.append((b, q0, min(CHUNK, L - q0), kb))
    # stable sort by descending key-block count
    chunks.sort(key=lambda c: -c[3])
    bigs = chunks[:B]
    smalls = chunks[B:]
    assert len(smalls) <= B, "more than 16 query chunks cannot happen (lens<=1024)"
    big_kb = max([c[3] for c in bigs], default=1)
    small_kb = max([c[3] for c in smalls], default=1)
    s = _Sched()
    s.lens = lens
    s.bigs = bigs + [None] * (B - len(bigs))
    s.smalls = smalls + [None] * (B - len(smalls))
    s.big_kb = max(4, big_kb)
    s.small_kb = max(4, small_kb)
    return s


# ---------------------------------------------------------------------------
# device program
# ---------------------------------------------------------------------------
def _build_nc(big_kb, small_kb):
    import bass_rust
    import concourse.bass as bass
    import concourse.tile as tile
    import concourse.mybir as mybir
    from concourse.vector_clock import ScopedClock

    # ---- workaround: this walrus build accepts at most ONE sem wait per
    # instruction ("Too many sync wait commands" in setupSyncWait). Tile
    # attaches multi-sem waits freely. Split: hoist all but the last wait of
    # every committed instruction onto injected same-engine NOPs, and split
    # the final drain the same way.
    _ctr = [0]

    def _hoist_excess_waits(tc_self, inst, orig_add):
        si = inst.sync_info
        if si is not None:
            waits = list(si.on_wait or [])
            if len(waits) > 1:
                for w in waits[:-1]:
                    _ctr[0] += 1
                    nop = mybir.InstNoOp(name=f"waitsplit-{_ctr[0]}")
                    nop.engine = inst.engine
                    nop.sync_info = bass_rust.SyncInfo(on_wait=[w], on_update=[])
                    orig_add(tc_self, nop)
                si.on_wait = waits[-1:]
        orig_add(tc_self, inst)

    if not getattr(tile.TileContext, "_waitsplit_patched", False):
        _orig_add_instruction = tile.TileContext._add_instruction

        def _split_add_instruction(self, inst):
            _hoist_excess_waits(self, inst, _orig_add_instruction)

        tile.TileContext._add_instruction = _split_add_instruction

        def _patched_drain_and_barrier(self, tick_clock, wait_clock):
            nc = self.nc
            d0 = nc.sync.drain()
            wait_clock.add_sem_waits(
                d0.ins, ScopedClock({None: tick_clock.global_clock})
            )
            si = d0.ins.sync_info
            waits = list(si.on_wait) if si is not None else []
            if len(waits) > 1:
                si.on_wait = waits[0:1]
                for w in waits[1:]:
                    dk = nc.sync.drain()
                    dk.ins.sync_info = bass_rust.SyncInfo(on_wait=[w], on_update=[])
            nc.all_engine_barrier()
            assert self.sems is not None
            popped = nc._tile_sem_poison_stack.pop()
            assert popped is self._sem_poison
            nc.clear_and_free_semaphores(list(self.sems.allocated().values()))
            nc.all_engine_barrier()

        tile.TileContext._drain_and_barrier = _patched_drain_and_barrier
        tile.TileContext._waitsplit_patched = True

    dt = mybir.dt
    f32 = dt.float32
    f32r = dt.float32r
    bf16 = dt.bfloat16
    AF = mybir.ActivationFunctionType

    KBS = (big_kb, small_kb)
    KBTOT = big_kb + small_kb
    KT = KBTOT * 128  # kt/xk columns per channel-tile
    REG0 = big_kb * 128  # region-0 key columns

    nc = bass.Bass(
        "TRN2",
        target_bir_lowering=False,
        debug=False,
        num_devices=1,
        enable_asserts=False,
    )

    xk_d = nc.dram_tensor("xk", [128, 2 * KT], f32r, kind="ExternalInput").ap()
    qw_d = nc.dram_tensor("qw", [128, 1536], f32r, kind="ExternalInput").ap()
    qb_d = nc.dram_tensor("qb", [128, 2], f32, kind="ExternalInput").ap()
    pw_d = nc.dram_tensor("pw", [128, 512], f32r, kind="ExternalInput").ap()
    pb_d = nc.dram_tensor("pb", [128, 2], f32, kind="ExternalInput").ap()
    mask_d = nc.dram_tensor("mask", [128, KBTOT], f32, kind="ExternalInput").ap()
    onesb_d = nc.dram_tensor("onesb", [128, 4], bf16, kind="ExternalInput").ap()
    sel_d = nc.dram_tensor("sel", [4, 128], f32, kind="ExternalInput").ap()
    yt_d = nc.dram_tensor("yt", [128, 2048], f32, kind="ExternalOutput").ap()

    with tile.TileContext(nc) as tc:
        with (
            tc.tile_pool(name="const", bufs=1) as const_pool,
            tc.tile_pool(name="big", bufs=1) as big_pool,
            tc.tile_pool(name="pt", bufs=20) as pt_pool,
            tc.tile_pool(name="stage", bufs=4) as stage_pool,
            tc.tile_pool(name="norm", bufs=2) as norm_pool,
            tc.tile_pool(name="ps_s4a", bufs=1, space="PSUM") as ps_s4a,
            tc.tile_pool(name="ps_s4b", bufs=1, space="PSUM") as ps_s4b,
            tc.tile_pool(name="ps_acc", bufs=2, space="PSUM") as ps_acc,
        ):
            xk = const_pool.tile([128, 2 * KT], f32r, tag="xk")
            qw = const_pool.tile([128, 1536], f32r, tag="qw")
            qb = const_pool.tile([128, 2], f32, tag="qb")
            pw = const_pool.tile([128, 512], f32r, tag="pw")
            pb = const_pool.tile([128, 2], f32, tag="pb")
            mask = const_pool.tile([128, KBTOT], f32, tag="mask")
            onesb = const_pool.tile([128, 4], bf16, tag="onesb")
            sel = const_pool.tile([4, 128], f32, tag="sel")

            # DMA plan: the first exp is gated on qw + the first 512 key
            # cols of both channel-tiles, so those issue first on their
            # queues. The scalar queue is used only during the idle head.
            nc.gpsimd.dma_start(sel[:], sel_d)
            warm = const_pool.tile([4, 128], f32, tag="warm")
            nc.scalar.activation(warm[:], sel[:], AF.Exp, scale=0.0)
            nc.sync.dma_start(qw[:, 0:768], qw_d[:, 0:768])
            nc.scalar.dma_start(qw[:, 768:1536], qw_d[:, 768:1536])
            nc.gpsimd.dma_start(mask[:], mask_d)
            nc.gpsimd.dma_start(qb[:], qb_d)
            for c0 in range(0, REG0, 512):
                ce = min(c0 + 512, REG0)
                nc.sync.dma_start(xk[:, c0:ce], xk_d[:, c0:ce])
                nc.gpsimd.dma_start(xk[:, KT + c0 : KT + ce], xk_d[:, KT + c0 : KT + ce])
            for c0 in range(REG0, KT, 512):
                ce = min(c0 + 512, KT)
                nc.sync.dma_start(xk[:, c0:ce], xk_d[:, c0:ce])
                nc.scalar.dma_start(xk[:, KT + c0 : KT + ce], xk_d[:, KT + c0 : KT + ce])
            nc.gpsimd.dma_start(onesb[:], onesb_d)
            nc.gpsimd.dma_start(pb[:], pb_d)
            nc.gpsimd.dma_start(pw[:], pw_d)

            qt = big_pool.tile([128, 2048], f32r, tag="qt")
            kt = big_pool.tile([128, 2 * KT], f32r, tag="kt")
            va = big_pool.tile([128, KBTOT, 8, 32], bf16, tag="va")
            otf = big_pool.tile([128, 2048], f32r, tag="otf")
            ytile = big_pool.tile([128, 2048], f32, tag="ytile")

            _acc_flip = [0]

            def acc_tile(shape, dtype, name):
                tag = "acca" if _acc_flip[0] == 0 else "accb"
                _acc_flip[0] ^= 1
                return ps_acc.tile(shape, dtype, tag=tag, name=name)

            # ---------- projection scratch (emitted before any PV binds) ----
            def q_tile(m, c):
                ps = acc_tile([128, 512], f32, f"q{m}{c}")
                qoff = REG0 if c == 1 else 0
                for t in range(2):
                    nc.tensor.matmul(
                        ps[:],
                        qw[:, 768 * t + 128 * m : 768 * t + 128 * (m + 1)],
                        xk[:, KT * t + qoff : KT * t + qoff + 512],
                        start=(t == 0),
                        stop=(t == 1),
                    )
                nc.vector.tensor_scalar_add(
                    qt[:, 1024 * m + 512 * c : 1024 * m + 512 * (c + 1)],
                    ps[:],
                    qb[:, m : m + 1],
                )

            def k_tile(m2, c0, w):
                ps = acc_tile([128, 512], f32, f"k{m2}{c0}")
                for t in range(2):
                    nc.tensor.matmul(
                        ps[0:128, 0:w],
                        qw[:, 768 * t + 128 * (2 + m2) : 768 * t + 128 * (3 + m2)],
                        xk[:, KT * t + c0 : KT * t + c0 + w],
                        start=(t == 0),
                        stop=(t == 1),
                    )
                nc.vector.tensor_copy(kt[:, KT * m2 + c0 : KT * m2 + c0 + w], ps[0:128, 0:w])

            def v_tile(j):
                ps = acc_tile([128, 512], f32, f"v{j}")
                for t in range(2):
                    nc.tensor.matmul(
                        ps[0:128, 0:256],
                        xk[:, KT * t + 128 * j : KT * t + 128 * (j + 1)],
                        qw[:, 768 * t + 512 : 768 * t + 768],
                        start=(t == 0),
                        stop=(t == 1),
                    )
                nc.vector.tensor_copy(
                    va[:, j, :, :],
                    ps[0:128, 0:256].rearrange("p (h d) -> p h d", d=32),
                )

            # per-iteration scratch emission plan (all before the first PV)
            kchunks = [(c0, min(512, KT - c0)) for c0 in range(0, KT, 512)]
            scratch = {None: [lambda: k_tile(0, *kchunks[0]), lambda: q_tile(0, 0)]}
            rest = []
            if len(kchunks) > 1:
                rest.append(lambda: k_tile(0, *kchunks[1]))
            rest += [lambda: v_tile(0), lambda: v_tile(1), lambda: q_tile(0, 1)]
            rest += [lambda kc=kc: k_tile(1, *kc) for kc in kchunks[:2]]
            rest += [lambda: v_tile(2), lambda: v_tile(3)]
            rest += [lambda kc=kc: k_tile(0, *kc) for kc in kchunks[2:]]
            rest += [lambda: v_tile(4), lambda: v_tile(5), lambda: q_tile(1, 0)]
            rest += [lambda kc=kc: k_tile(1, *kc) for kc in kchunks[2:]]
            rest += [lambda: q_tile(1, 1)]
            rest += [lambda j=j: v_tile(j) for j in range(6, KBTOT)]
            LAG = 5
            per = max(1, math.ceil(len(rest) / LAG))
            for i in range(LAG):
                scratch[i] = rest[i * per : (i + 1) * per]

            # ---------- attention loop ----------
            iters = [
                (g, s, j) for g in range(2) for s in range(2) for j in range(KBS[s])
            ]

            def emit_scores(g, s, j):
                jj = j + (big_kb if s == 1 else 0)
                s4a = ps_s4a.tile([128, 1024], f32, tag="s4a", name=f"s4a{g}{s}{j}")
                s4b = ps_s4b.tile([128, 1024], f32, tag="s4b", name=f"s4b{g}{s}{j}")
                for hh in range(4):
                    s4 = s4a if hh < 2 else s4b
                    base = 32 * hh
                    nc.tensor.matmul(
                        s4[:, 512 * (hh % 2) : 512 * (hh % 2 + 1)],
                        kt[base : base + 32, KT * g + 128 * jj : KT * g + 128 * (jj + 1)],
                        qt[base : base + 32, 1024 * g + 512 * s : 1024 * g + 512 * (s + 1)],
                        start=True,
                        stop=True,
                        tile_position=(base, 0),
                    )
                pta = pt_pool.tile([128, 1024], bf16, tag="pt", name=f"pta{g}{s}{j}")
                ptb = pt_pool.tile([128, 1024], bf16, tag="pt", name=f"ptb{g}{s}{j}")
                nc.scalar.activation(
                    pta[:], s4a[:], AF.Exp, bias=mask[:, jj : jj + 1], scale=SCALE
                )
                nc.scalar.activation(
                    ptb[:], s4b[:], AF.Exp, bias=mask[:, jj : jj + 1], scale=SCALE
                )
                return pta, ptb

            accs = {}  # (g, s) -> (ova, ovb)

            def emit_pv_den(g, s, j, pta, ptb):
                if j == 0:
                    ova = acc_tile([128, 512], f32, f"ova{g}{s}")
                    ovb = acc_tile([128, 512], f32, f"ovb{g}{s}")
                    accs[(g, s)] = (ova, ovb)
                ova, ovb = accs[(g, s)]
                jj = j + (big_kb if s == 1 else 0)
                sj = j == 0
                ej = j == KBS[s] - 1
                for hh in range(4):
                    ov = ova if hh < 2 else ovb
                    pt = pta if hh < 2 else ptb
                    vpos = 32 * hh
                    nc.tensor.matmul(
                        ov[vpos : vpos + 32, :],
                        va[:, jj, 4 * g + hh, :],
                        pt[:, 512 * (hh % 2) : 512 * (hh % 2 + 1)],
                        start=sj,
                        stop=ej,
                        tile_position=(0, vpos),
                    )
                for hh in range(4):
                    ov = ova if hh < 2 else ovb
                    pt = pta if hh < 2 else ptb
                    spos = (64, 96, 0, 32)[hh]
                    nc.tensor.matmul(
                        ov[spos : spos + 1, :],
                        onesb[:, 0:1],
                        pt[:, 512 * (hh % 2) : 512 * (hh % 2 + 1)],
                        start=sj,
                        stop=ej,
                        tile_position=(0, spos),
                    )

            def emit_norm(g, s, last=False):
                """Gather denominators, broadcast reciprocals, write otf strips.
                Non-last norms stage the psum accumulators to sbuf so the acc
                slots recycle; the last norm reads psum directly."""
                ova, ovb = accs.pop((g, s))
                sta = stage_pool.tile([128, 512], f32, tag="st", name=f"sta{g}{s}")
                stb = stage_pool.tile([128, 512], f32, tag="st", name=f"stb{g}{s}")
                # denominator rows staged first so the se4 gather can issue;
                # the strip rows copy during the gather + reciprocal
                nc.vector.tensor_copy(sta[64:97, :], ova[64:97, :])
                nc.vector.tensor_copy(stb[0:33, :], ovb[0:33, :])
                se4 = norm_pool.tile([4, 512], f32, tag="se4", name=f"se4{g}{s}")
                nc.gpsimd.dma_start(se4[0:1, :], sta[64:65, :])
                nc.sync.dma_start(se4[1:2, :], sta[96:97, :])
                nc.gpsimd.dma_start(se4[2:3, :], stb[0:1, :])
                nc.sync.dma_start(se4[3:4, :], stb[32:33, :])
                nc.vector.tensor_copy(sta[0:64, :], ova[0:64, :])
                nc.vector.tensor_copy(stb[64:128, :], ovb[64:128, :])
                ln4 = norm_pool.tile([4, 512], f32, tag="ln4", name=f"ln4{g}{s}")
                rc4 = norm_pool.tile([4, 512], f32, tag="rc4", name=f"rc4{g}{s}")
                nc.scalar.activation(ln4[:], se4[:], AF.Ln)
                nc.scalar.activation(rc4[:], ln4[:], AF.Exp, scale=-1.0)
                if last:
                    bc = ps_s4a.tile([128, 512], f32, tag="s4a", name=f"bc{g}{s}")
                else:
                    bc = acc_tile([128, 512], f32, f"bc{g}{s}")
                nc.tensor.matmul(bc[:], sel[:], rc4[:], start=True, stop=True)
                col = 1024 * g + 512 * s
                nc.vector.tensor_mul(
                    otf[0:64, col : col + 512], sta[0:64, :], bc[0:64, :]
                )
                nc.vector.tensor_mul(
                    otf[64:128, col : col + 512], stb[64:128, :], bc[64:128, :]
                )

            def emit_proj(c):
                for m in range(2):
                    ps = acc_tile([128, 512], f32, f"y{m}{c}")
                    for t in range(2):
                        nc.tensor.matmul(
                            ps[:],
                            pw[:, 256 * t + 128 * m : 256 * t + 128 * (m + 1)],
                            otf[:, 1024 * t + 512 * c : 1024 * t + 512 * (c + 1)],
                            start=(t == 0),
                            stop=(t == 1),
                        )
                    col = 1024 * m + 512 * c
                    nc.vector.tensor_scalar_add(
                        ytile[:, col : col + 512], ps[:], pb[:, m : m + 1]
                    )
                    # keep ScalarE exp-only until the last exp has issued:
                    # proj(c=0) runs while group-1 exps are still streaming
                    eng = nc.sync if m == 0 else (nc.scalar if c == 1 else nc.gpsimd)
                    eng.dma_start(
                        yt_d[:, col : col + 512], ytile[:, col : col + 512]
                    )

            for t in scratch[None]:
                t()
            pts = {}
            for idx in range(len(iters) + LAG):
                if idx < len(iters):
                    pts[idx] = emit_scores(*iters[idx])
                    for t in scratch.get(idx, []):
                        t()
                lag_idx = idx - LAG
                if 0 <= lag_idx < len(iters):
                    g, s, j = iters[lag_idx]
                    emit_pv_den(g, s, j, *pts.pop(lag_idx))
                    if j == KBS[s] - 1 and not (g == 1 and s == 1):
                        emit_norm(g, s)
            emit_proj(0)
            emit_norm(1, 1, last=True)
            emit_proj(1)

    return nc


def _get_nc(big_kb, small_kb):
    key = (big_kb, small_kb)
    if key not in _cached:
        _cached[key] = _build_nc(big_kb, small_kb)
    return _cached[key]


def _pack_per_partition(a2d):
    """[2*128, F] -> [128, 2*F] with tile t at cols F*t."""
    n, f = a2d.shape
    t = n // 128
    return np.ascontiguousarray(
        a2d.reshape(t, 128, f).transpose(1, 0, 2).reshape(128, t * f)
    )


def _prepare(carrier_tokens, ct_mask, batch_num_windows, qkv_w, qkv_b, proj_w, proj_b):
    """Host-side bookkeeping: chunk scheduling, gathers, weight packing."""
    carrier_tokens = np.asarray(carrier_tokens, dtype=np.float32)
    ct_mask = np.asarray(ct_mask, dtype=np.float32)
    lens_raw = np.asarray(batch_num_windows).astype(np.int64)
    qkv_w = np.asarray(qkv_w, dtype=np.float32)
    qkv_b = np.asarray(qkv_b, dtype=np.float32)
    proj_w = np.asarray(proj_w, dtype=np.float32)
    proj_b = np.asarray(proj_b, dtype=np.float32)

    total = carrier_tokens.shape[0]
    sched = _schedule(lens_raw)
    lens = sched.lens
    KBTOT = sched.big_kb + sched.small_kb
    KT = KBTOT * 128

    offsets = np.concatenate([[0], np.cumsum(lens_raw)])
    mask_col = np.ascontiguousarray(ct_mask[:, 0, :])  # [B, MAXW]

    # batch key-token matrices, zero-padded to MAXW
    xb = np.zeros((B, MAXW, C), np.float32)
    for b in range(B):
        L = int(min(lens[b], max(0, total - offsets[b])))
        if L > 0:
            xb[b, :L] = carrier_tokens[offsets[b] : offsets[b] + L]

    # host-side exact weight transforms
    pb_eff = qkv_b[2 * C : 3 * C] @ proj_w + proj_b
    qw_packed = _pack_per_partition(qkv_w)  # [128, 1536]
    qb_packed = np.ascontiguousarray(qkv_b[0:C].reshape(2, 128).T)
    pw_packed = _pack_per_partition(proj_w)  # [128, 512]
    pb_packed = np.ascontiguousarray(pb_eff.reshape(2, 128).T)

    import ml_dtypes

    onesb_arr = np.ones((128, 4), ml_dtypes.bfloat16)
    sel_arr = np.zeros((4, 128), np.float32)
    for k in range(4):
        sel_arr[k, 32 * k : 32 * (k + 1)] = 1.0

    in_maps = []
    for i in range(B):
        slots = (sched.bigs[i], sched.smalls[i])
        widths = (sched.big_kb * 128, sched.small_kb * 128)
        # keys: one region per slot, cyclically rotated so the slot's 512
        # query tokens sit in the region's first columns (the device reads
        # Q from the region front; key order is irrelevant to attention)
        xk2 = np.zeros((KT, C), np.float32)
        kmask = np.full((128, KBTOT), -1e9, np.float32)
        kbase = 0
        jbase = 0
        for si, sl in enumerate(slots):
            w = widths[si]
            if sl is not None:
                b, q0, qlen, kb = sl
                reg = min(w, MAXW)
                perm = np.concatenate([np.arange(q0, reg), np.arange(0, q0)])
                xk2[kbase : kbase + reg] = xb[b, perm]
                m = mask_col[b, perm].reshape(-1, 128).T  # [128, reg/128]
                kmask[:, jbase : jbase + reg // 128] = m
            kbase += w
            jbase += w // 128
        in_maps.append(
            {
                "xk": _pack_per_partition(xk2.T),
                "qw": qw_packed,
                "qb": qb_packed,
                "pw": pw_packed,
                "pb": pb_packed,
                "mask": np.ascontiguousarray(kmask),
                "onesb": onesb_arr,
                "sel": sel_arr,
            }
        )

    ctx = {
        "sched": sched,
        "offsets": offsets,
        "total": total,
        "mask_col": mask_col,
        "xb": xb,
        "qkv_w": qkv_w,
        "qkv_b": qkv_b,
        "proj_w": proj_w,
        "proj_b": proj_b,
    }
    return in_maps, ctx


def _host_zero_rows(ctx, batches):
    """Exact reference rows for padded-position gathers / fully-masked
    batches: attention output for each batch's padded (zero) query token,
    with the reference's max-subtracted softmax."""
    out = {}
    qkv_w, qkv_b = ctx["qkv_w"], ctx["qkv_b"]
    proj_w, proj_b = ctx["proj_w"], ctx["proj_b"]
    mask_col = ctx["mask_col"]
    for b in batches:
        xpad = ctx["xb"][b]  # [MAXW, C] zero-padded
        kmat = xpad @ qkv_w[:, C : 2 * C] + qkv_b[C : 2 * C]
        vmat = xpad @ qkv_w[:, 2 * C : 3 * C] + qkv_b[2 * C : 3 * C]
        qvec = qkv_b[0:C].reshape(H, HD) * SCALE
        kh = kmat.reshape(MAXW, H, HD)
        sc = np.einsum("hd,khd->hk", qvec, kh) + mask_col[b][None, :]
        sc = sc - sc.max(axis=1, keepdims=True)
        e = np.exp(sc)
        attn = e / e.sum(axis=1, keepdims=True)
        ct = np.einsum("hk,khd->hd", attn, vmat.reshape(MAXW, H, HD)).reshape(C)
        out[b] = ct @ proj_w + proj_b
    return out


def _postprocess(results, ctx):
    """Per-core outputs -> full ragged output."""
    sched = ctx["sched"]
    total = ctx["total"]
    offsets = ctx["offsets"]
    mask_col = ctx["mask_col"]

    y_pad = np.zeros((B * MAXW, C), np.float32)
    written = np.zeros(B * MAXW, bool)
    for i in range(B):
        yt = results[i]["yt"]  # [128, 2048]
        y_t = yt.reshape(128, 2, 1024).transpose(1, 0, 2).reshape(C, 1024)
        y_core = y_t.T  # [1024, C]
        for si, sl in enumerate((sched.bigs[i], sched.smalls[i])):
            if sl is None:
                continue
            b, q0, qlen, _ = sl
            y_pad[b * MAXW + q0 : b * MAXW + q0 + qlen] = y_core[
                512 * si : 512 * si + qlen
            ]
            written[b * MAXW + q0 : b * MAXW + q0 + qlen] = True

    # reference gather semantics (OOB scatter dropped, OOB gather clipped)
    tok = np.arange(total)
    b_id = np.searchsorted(offsets[1:], tok, side="right")
    w_id = tok - offsets[np.minimum(b_id, B)]
    flat_idx = b_id * MAXW + w_id
    gather_idx = np.clip(flat_idx, 0, B * MAXW - 1)
    out = y_pad[gather_idx]

    # rare exact fixes: gathers landing on never-computed padded rows, and
    # fully-masked batches (device exp gives 0/0 where the reference's
    # max-subtracted softmax gives uniform weights)
    degenerate = set(
        b for b in range(B) if np.all(mask_col[b] < NEG_THRESH)
    )
    bad = ~written[gather_idx]
    row_b = np.minimum(gather_idx // MAXW, B - 1)
    if degenerate:
        bad |= np.isin(row_b, list(degenerate))
    if bad.any():
        need = set(row_b[bad].tolist()) | degenerate
        fixes = _host_zero_rows(ctx, sorted(need))
        for b, fix in fixes.items():
            rows = np.nonzero(bad & (row_b == b))[0]
            out[rows] = fix.astype(np.float32)

    return np.ascontiguousarray(out.astype(np.float32))


def run_device(in_maps, **spmd_kwargs):
    from concourse import bass_utils

    nc = _cached["last_nc"]
    return bass_utils.run_bass_kernel_spmd(
        nc, in_maps, core_ids=list(range(B)), **spmd_kwargs
    )


def kernel(carrier_tokens, ct_mask, batch_num_windows, qkv_w, qkv_b, proj_w, proj_b):
    in_maps, ctx = _prepare(
        carrier_tokens, ct_mask, batch_num_windows, qkv_w, qkv_b, proj_w, proj_b
    )
    sched = ctx["sched"]
    _cached["last_nc"] = _get_nc(sched.big_kb, sched.small_kb)
    res = run_device(in_maps, trace=False)
    return _postprocess(res.results, ctx)
